# revision 8
# baseline (speedup 1.0000x reference)
"""Binomial deviance loss on 8 Trainium2 NeuronCores (Bass/Tile, SPMD-free).

loss = sum(w * log1p(exp(-ALPHA*(S-BETA)*m))),  S = triu(cosine(x_i, y_j)),
ALPHA = 2.0, BETA = 0.5  ->  t = (1 - 2*S)*m and loss = sum(w * softplus(t)).

Sharding: the 64 128-row tiles of x/m/w are dealt round-robin across the 8
NeuronCores (core c gets tiles {8g+c}), which balances the triangular-mask
work exactly; y is replicated. m and w are converted to bf16 on the host and
concatenated row-tile-wise into one [RPC, 2N] array so each 128-row tile is
a single 4 MB DMA (m in cols [0,N), w in cols [N,2N)) — this halves the HBM
traffic, which is the roofline for this problem. Each core returns a scalar
partial loss [1, 1]; the host sums the 8 scalars.

Per core c, local row-tile g (global 128-row tile G = 8g+c), chunks of 1024:
  x_hat = x * (-2/||x||)   (so the PE output is v = -2S)
  y_hat = y * (1/||y||)
  chunk k < g : fully masked -> t = m (ACT exp reads m directly)
  chunk k > g : v = PE(x_hatT, y_hatT) in fp32 PSUM;  t = (v+1)*m   (DVE STT)
  chunk k == g: cols < c*128 zero-filled in PSUM by a 1-partition matmul,
                128-block c gets the triangular mask (DVE TT with tri), then
                t = (v+1)*m as above.
  ACT (row-batched, 3 instructions per row-tile):
      tbuf[:, :g*1024]  = exp(m_prefix)        (masked part, from m directly)
      tbuf[:, g*1024:]  = exp(tbuf suffix)     (in place)
      tbuf              = ln(tbuf + 1)         (in place -> softplus)
  prod = w .* tbuf per chunk (bf16 TT, 2x mode), then a 1-partition ones
  matmul accumulates column sums into a persistent [1, 512] PSUM bank; a
  single tensor_reduce after the loop yields the core's scalar.

Each module is compiled once per process and relaunched for repeat calls.
"""

import threading
from contextlib import ExitStack

import numpy as np
import ml_dtypes

import concourse.bass as bass
import concourse.tile as tile
import concourse.mybir as mybir

N = 8192
D = 128
NC = 8
RPC = N // NC          # rows per core
NRT = RPC // 128       # 128-row tiles per core
CW = 1024              # compute chunk width (columns)
NK = N // CW           # chunks per row-tile
MW_BUFS = 3

F32 = mybir.dt.float32
BF16 = mybir.dt.bfloat16
AF = mybir.ActivationFunctionType
ALU = mybir.AluOpType


def _install_drain_patch():
    """The walrus build in this container rejects more than a couple of sem
    waits on one instruction; the Tile tail drain carries one wait per live
    semaphore. Emit them as individual sync-engine WAIT instructions."""
    if getattr(tile.TileContext, "_drain_patched", False):
        return

    def _patched(self, tick_clock, wait_clock):
        nc = self.nc
        carrier = nc.sync.nop()
        wait_clock.add_sem_waits(
            carrier.ins, tile.ScopedClock({None: tick_clock.global_clock})
        )
        si = carrier.ins.sync_info
        waits = list(si.on_wait or []) if si is not None else []
        if si is not None:
            si.on_wait = []
        handles = {}
        for h in self.sems.allocated().values():
            handles[getattr(h, "name", None) or getattr(h, "ant_name", None)] = h
        for w in waits:
            nc.sync.wait_ge(handles[w.ant_name], w.wait_value)
        nc.sync.drain()
        nc.all_engine_barrier()
        popped = nc._tile_sem_poison_stack.pop()
        assert popped is self._sem_poison
        nc.clear_and_free_semaphores(list(self.sems.allocated().values()))
        nc.all_engine_barrier()

    tile.TileContext._drain_and_barrier = _patched
    tile.TileContext._drain_patched = True


def _legalize_waits(nc, maxw=1):
    """Hoist excess per-instruction sem waits onto standalone EventSemaphore
    carriers just before the instruction on the same engine (wait A; wait B;
    inst  ==  inst waiting on A AND B)."""
    for fn in nc.m.functions:
        for blk in fn.blocks:
            insts = list(blk.instructions)
            new = []
            for inst in insts:
                si = inst.sync_info
                waits = list(si.on_wait) if si is not None and si.on_wait else []
                if len(waits) > maxw:
                    for i, w in enumerate(waits[:-maxw]):
                        new.append(mybir.InstEventSemaphore(
                            name=f"{inst.name}_hw{i}",
                            engine=inst.engine,
                            ins=[],
                            outs=[],
                            sync_info=mybir.SyncInfo(on_wait=[w], on_update=[]),
                        ))
                    si.on_wait = waits[-maxw:]
                new.append(inst)
            if len(new) != len(insts):
                blk.instructions[:] = new


def build_core_module(c: int, iters: int = 1, mw_dma_eng: str = "sync",
                      mw_bufs: int = MW_BUFS, psum_bufs: int = 3,
                      tbuf_bufs: int = 3, prod_fd: int = 512,
                      gp_mod: int = 2) -> bass.Bass:
    _install_drain_patch()
    nc = bass.Bass("TRN2", target_bir_lowering=False, debug=False)

    xs = nc.dram_tensor("xs", [RPC, D], F32, kind="ExternalInput").ap()
    y = nc.dram_tensor("y", [N, D], F32, kind="ExternalInput").ap()
    mws = nc.dram_tensor("mws", [RPC, 2 * N], BF16, kind="ExternalInput").ap()
    out = nc.dram_tensor("out", [1, 1], F32, kind="ExternalOutput").ap()

    ident_np = np.eye(128, dtype=np.float32)
    tri_np = (np.arange(128)[None, :] >= np.arange(128)[:, None]).astype(np.float32)
    ident_dram = nc.inline_tensor(ident_np, name="ident").ap()
    tri_dram = nc.inline_tensor(tri_np, name="tri").ap()

    with tile.TileContext(nc) as tc, ExitStack() as ctx:
        consts = ctx.enter_context(tc.tile_pool(name="consts", bufs=1))
        pre = ctx.enter_context(tc.tile_pool(name="pre", bufs=2))
        smalls = ctx.enter_context(tc.tile_pool(name="smalls", bufs=4))
        persist = ctx.enter_context(tc.tile_pool(name="persist", bufs=1))
        mw = ctx.enter_context(tc.tile_pool(name="mw", bufs=mw_bufs))
        tb = ctx.enter_context(tc.tile_pool(name="tb", bufs=tbuf_bufs))
        prodw = ctx.enter_context(tc.tile_pool(name="prodw", bufs=3))
        psum_mm = ctx.enter_context(tc.tile_pool(name="psum_mm", bufs=psum_bufs,
                                                 space="PSUM"))
        psum_tr = ctx.enter_context(tc.tile_pool(name="psum_tr", bufs=1, space="PSUM"))
        psum_cs = ctx.enter_context(tc.tile_pool(name="psum_cs", bufs=1, space="PSUM"))

        ident_sb = consts.tile([128, 128], BF16, tag="ident")
        identf = pre.tile([128, 128], F32, tag="identf")
        nc.sync.dma_start(out=identf, in_=ident_dram)
        nc.vector.tensor_copy(out=ident_sb, in_=identf)
        tri_sb = consts.tile([128, 128], F32, tag="tri")
        nc.sync.dma_start(out=tri_sb, in_=tri_dram)

        ones_col = consts.tile([128, 1], BF16, tag="ones_col")
        nc.vector.memset(ones_col, 1.0)
        onesT1 = consts.tile([1, 128], BF16, tag="onesT1")
        nc.vector.memset(onesT1, 1.0)
        zrow = consts.tile([1, 512], BF16, tag="zrow")
        nc.vector.memset(zrow, 0.0)

        colsum = psum_cs.tile([1, 512], F32, tag="colsum")
        nc.tensor.matmul(colsum, onesT1[:, 0:1], zrow, start=True, stop=True)

        def prep_group(src_ap, g, dsts, negate):
            """Normalize + transpose rows [g*1024, (g+1)*1024) of src into
            eight [128,128] bf16 destination slices dsts[b]."""
            big = pre.tile([128, 8, 128], F32, tag="big")
            nc.sync.dma_start(
                out=big,
                in_=src_ap[g * 1024 : (g + 1) * 1024, :].rearrange(
                    "(b p) d -> p b d", p=128
                ),
            )
            sq = pre.tile([128, 8, 128], F32, tag="sq")
            nc.vector.tensor_mul(out=sq, in0=big, in1=big)
            n28 = smalls.tile([128, 8], F32, tag="n28")
            nc.vector.tensor_reduce(
                out=n28, in_=sq, axis=mybir.AxisListType.X, op=ALU.add
            )
            # rn = n2^(-1/2) via exp(-0.5*ln(n2))
            nc.scalar.activation(out=n28, in_=n28, func=AF.Ln)
            rn8 = smalls.tile([128, 8], F32, tag="rn8")
            nc.scalar.activation(out=rn8, in_=n28, func=AF.Exp, scale=-0.5)
            for b in range(8):
                hat = pre.tile([128, 128], BF16, tag="hat")
                if negate:
                    # x_hat = -2 * x / ||x||  so the PE produces v = -2S
                    nc.vector.tensor_scalar(
                        out=hat, in0=big[:, b, :], scalar1=rn8[:, b : b + 1],
                        scalar2=-2.0, op0=ALU.mult, op1=ALU.mult,
                    )
                else:
                    nc.vector.tensor_scalar_mul(
                        out=hat, in0=big[:, b, :], scalar1=rn8[:, b : b + 1]
                    )
                pt = psum_tr.tile([128, 128], BF16, tag="pt")
                nc.tensor.matmul(pt, hat, ident_sb, is_transpose=True)
                nc.vector.tensor_copy(out=dsts[b], in_=pt)

        xT = [persist.tile([128, 128], BF16, tag=f"xT{rt}", name=f"xT{rt}")
              for rt in range(NRT)]
        prep_group(xs, 0, xT, negate=True)

        yT = [persist.tile([128, CW], BF16, tag=f"yT{k}", name=f"yT{k}")
              for k in range(NK)]
        for k in range(NK):
            prep_group(y, k, [yT[k][:, b * 128 : (b + 1) * 128] for b in range(8)],
                       negate=False)

        def prod_phase(w_row, tbuf):
            """prod = w .* softplus (bf16 TT, 2x), column sums into PSUM via
            1-partition ones matmuls. Emitted one row-tile late so the PE's
            colsum matmuls (which wait on the end of a row's pipeline) queue
            behind the NEXT row's v matmuls rather than ahead of them.
            Every gp_mod-th chunk's multiply runs on the otherwise-idle GPSIMD
            engine to take load off the DVE; DVE chunks are split at prod_fd
            to shrink the post-op pipeline-drain penalty."""
            for k in range(NK):
                prod = prodw.tile([128, CW], BF16, tag="prod")
                if gp_mod > 0 and k % gp_mod == 0:
                    nc.gpsimd.tensor_mul(
                        out=prod, in0=w_row[:, k * CW : (k + 1) * CW],
                        in1=tbuf[:, k * CW : (k + 1) * CW],
                    )
                else:
                    for s in range(0, CW, prod_fd):
                        nc.vector.tensor_mul(
                            out=prod[:, s : s + prod_fd],
                            in0=w_row[:, k * CW + s : k * CW + s + prod_fd],
                            in1=tbuf[:, k * CW + s : k * CW + s + prod_fd],
                        )
                for s in range(0, CW, 512):
                    nc.tensor.matmul(colsum, ones_col, prod[:, s : s + 512],
                                     start=False, stop=True,
                                     skip_group_check=True)

        def main_loop():
            pending = None  # (w_row, tbuf) of the previous row-tile
            for g in range(NRT):
                mwbig = mw.tile([128, 2 * N], BF16, tag="mw")
                getattr(nc, mw_dma_eng).dma_start(
                    out=mwbig, in_=mws[g * 128 : (g + 1) * 128, :]
                )
                m_row = mwbig[:, 0:N]
                w_row = mwbig[:, N : 2 * N]

                tbuf = tb.tile([128, N], BF16, tag="tbuf")

                # ---- t for unmasked chunks (k >= g) via PE + STT ----
                for k in range(g, NK):
                    v = psum_mm.tile([128, CW], F32, tag="v")
                    if k == g:
                        # cols < c*128 are fully masked: zero-fill via a
                        # 1-partition matmul so t = (0+1)*m = m there
                        cmask = c * 128
                        for s in range(0, cmask, 512):
                            e = min(cmask, s + 512)
                            nc.tensor.matmul(v[:, s:e], onesT1, zrow[:, 0 : e - s],
                                             start=True, stop=True)
                        for s in range(0, CW, 512):
                            e = s + 512
                            s0 = max(s, cmask)
                            if s0 < e:
                                nc.tensor.matmul(
                                    v[:, s0:e], xT[g],
                                    yT[k][:, s0:e], start=True, stop=True,
                                )
                        # triangular mask on 128-block c (v = -2S there; below
                        # the diagonal force v=0 so t = m)
                        nc.vector.tensor_mul(
                            out=v[:, cmask : cmask + 128],
                            in0=v[:, cmask : cmask + 128],
                            in1=tri_sb,
                        )
                    else:
                        for s in range(0, CW, 512):
                            nc.tensor.matmul(
                                v[:, s : s + 512], xT[g],
                                yT[k][:, s : s + 512], start=True, stop=True,
                            )
                    nc.vector.scalar_tensor_tensor(
                        out=tbuf[:, k * CW : (k + 1) * CW], in0=v, scalar=1.0,
                        in1=m_row[:, k * CW : (k + 1) * CW],
                        op0=ALU.add, op1=ALU.mult,
                    )

                # ---- row-batched ACT: exp(prefix from m), exp(suffix), ln ----
                if g > 0:
                    nc.scalar.activation(out=tbuf[:, 0 : g * CW],
                                         in_=m_row[:, 0 : g * CW],
                                         func=AF.Exp, scale=1.0)
                nc.scalar.activation(out=tbuf[:, g * CW :], in_=tbuf[:, g * CW :],
                                     func=AF.Exp, scale=1.0)
                nc.scalar.activation(out=tbuf, in_=tbuf, func=AF.Ln, bias=1.0)

                if pending is not None:
                    prod_phase(*pending)
                pending = (w_row, tbuf)
            prod_phase(*pending)

        if iters == 1:
            main_loop()
        else:
            # timing mode: repeat the streaming loop on-device so dispatch
            # overhead amortizes out of wall-clock measurements; branch hints
            # keep the large body's back-edge IRAM-resident
            with tc.For_i(0, iters, 1, hint_engines=(
                mybir.EngineType.DVE, mybir.EngineType.Activation,
                mybir.EngineType.PE, mybir.EngineType.SP,
            )):
                main_loop()

        total = smalls.tile([1, 1], F32, tag="total")
        nc.vector.tensor_reduce(
            out=total, in_=colsum, axis=mybir.AxisListType.X, op=ALU.add
        )
        nc.sync.dma_start(out=out, in_=total)

    _legalize_waits(nc)
    return nc


class CoreRunner:
    """One jitted bass_exec per (module, device); compiled once, relaunchable."""

    def __init__(self, nc, device):
        import jax
        from concourse import bass2jax

        bass2jax.install_neuronx_cc_hook()
        self.nc = nc
        self.device = device
        self.partition_name = (
            nc.partition_id_tensor.name if nc.partition_id_tensor is not None else None
        )
        in_names, out_names, out_avals = [], [], []
        self.out_shapes = []
        for alloc in nc.m.functions[0].allocations:
            if not isinstance(alloc, mybir.MemoryLocationSet):
                continue
            name = alloc.memorylocations[0].name
            if alloc.kind == "ExternalInput":
                if name != self.partition_name:
                    in_names.append(name)
            elif alloc.kind == "ExternalOutput":
                out_names.append(name)
                shape = tuple(alloc.tensor_shape)
                dtype = mybir.dt.np(alloc.dtype)
                out_avals.append(jax.core.ShapedArray(shape, dtype))
                self.out_shapes.append((shape, dtype))
        self.in_names = in_names
        self.out_names = out_names
        n_params, n_outs = len(in_names), len(out_names)
        extra = [self.partition_name] if self.partition_name else []
        all_in_names = tuple(in_names + out_names + extra)
        donate = tuple(range(n_params, n_params + n_outs))
        out_avals_t = tuple(out_avals)

        def _body(*args):
            outs = bass2jax._bass_exec_p.bind(
                *args,
                out_avals=out_avals_t,
                in_names=all_in_names,
                out_names=tuple(out_names),
                lowering_input_output_aliases=(),
                sim_require_finite=True,
                sim_require_nnan=True,
                nc=nc,
            )
            return tuple(outs)

        self.jitted = jax.jit(_body, donate_argnums=donate, keep_unused=True)
        self._dev_inputs = None
        self._pid = []

    def upload(self, in_map, core_id=0):
        import jax

        self._dev_inputs = [
            jax.device_put(np.ascontiguousarray(in_map[name]), self.device)
            for name in self.in_names
        ]
        self._pid = (
            [jax.device_put(np.array([[core_id]], np.uint32), self.device)]
            if self.partition_name
            else []
        )

    def launch(self):
        import jax

        zeros = [
            jax.device_put(np.zeros(shape, dtype), self.device)
            for shape, dtype in self.out_shapes
        ]
        return self.jitted(*self._dev_inputs, *zeros, *self._pid)

    def gather(self, outs):
        return {name: np.asarray(o) for name, o in zip(self.out_names, outs)}


_runners = None
_runner_lock = threading.Lock()


def get_runners():
    global _runners
    with _runner_lock:
        if _runners is None:
            import jax

            devs = jax.devices()
            assert len(devs) >= NC, f"need {NC} devices, have {len(devs)}"
            _runners = [CoreRunner(build_core_module(c), devs[c]) for c in range(NC)]
        return _runners


def shard_inputs(x, y, m, w):
    """Row-tiles (128 rows each) are dealt round-robin: core c gets global
    tiles {8g + c}, so every core sees the same balanced mix of masked /
    diagonal / unmasked column chunks (the diagonal of tile 8g+c falls in
    column chunk g for every core). m and w are cast to bf16 and concatenated
    per row-tile so each [128, 16384] tile is one contiguous 4 MB DMA."""
    bf16 = ml_dtypes.bfloat16
    xr = x.reshape(NC * NRT, 128, D)
    mr = np.asarray(m, dtype=np.float32).astype(bf16).reshape(NC * NRT, 128, N)
    wr = np.asarray(w, dtype=np.float32).astype(bf16).reshape(NC * NRT, 128, N)
    maps = []
    for c in range(NC):
        mw = np.concatenate([mr[c::NC], wr[c::NC]], axis=2).reshape(RPC, 2 * N)
        maps.append({
            "xs": xr[c::NC].reshape(RPC, D),
            "y": y,
            "mws": mw,
        })
    return maps


def kernel(x, y, m, w):
    x = np.asarray(x, dtype=np.float32)
    y = np.asarray(y, dtype=np.float32)
    m = np.asarray(m, dtype=np.float32)
    w = np.asarray(w, dtype=np.float32)
    assert x.shape == (N, D) and y.shape == (N, D)
    assert m.shape == (N, N) and w.shape == (N, N)
    runners = get_runners()
    maps = shard_inputs(x, y, m, w)
    for c, r in enumerate(runners):
        r.upload(maps[c], core_id=c)
    handles = [r.launch() for r in runners]
    results = [r.gather(h) for r, h in zip(runners, handles)]
    total = np.float64(0.0)
    for res in results:
        total += np.float64(res["out"].ravel()[0])
    return np.float32(total)


# revision 20
# speedup vs baseline: 1.2060x; 1.2060x over previous
"""Binomial deviance loss on 8 Trainium2 NeuronCores (Bass/Tile, SPMD-free).

loss = sum(w * log1p(exp(-ALPHA*(S-BETA)*m))),  S = triu(cosine(x_i, y_j)),
ALPHA = 2.0, BETA = 0.5  ->  t = (1 - 2*S)*m and loss = sum(w * softplus(t)).

Sharding: the 64 128-row tiles of x/m/w are dealt round-robin across the 8
NeuronCores (core c gets tiles {8g+c}), which balances the triangular-mask
work exactly; y is replicated. m and w are converted to bf16 on the host and
concatenated row-tile-wise into one [RPC, 2N] array so each 128-row tile is
a single 4 MB DMA (m in cols [0,N), w in cols [N,2N)) — this halves the HBM
traffic, which is the roofline for this problem. Each core returns a scalar
partial loss [1, 1]; the host sums the 8 scalars.

Per core c, local row-tile g (global 128-row tile G = 8g+c), chunks of 1024:
  x_hat = x * (-2/||x||)   (so the PE output is v = -2S)
  y_hat = y * (1/||y||)
  chunk k < g : fully masked -> t = m (ACT exp reads m directly)
  chunk k > g : v = PE(x_hatT, y_hatT) in fp32 PSUM;  t = (v+1)*m   (DVE STT)
  chunk k == g: cols < c*128 zero-filled in PSUM by a 1-partition matmul,
                128-block c gets the triangular mask (DVE TT with tri), then
                t = (v+1)*m as above.
  ACT (row-batched, 3 instructions per row-tile):
      tbuf[:, :g*1024]  = exp(m_prefix)        (masked part, from m directly)
      tbuf[:, g*1024:]  = exp(tbuf suffix)     (in place)
      tbuf              = ln(tbuf + 1)         (in place -> softplus)
  prod = w .* tbuf per chunk (bf16 TT, 2x mode), then a 1-partition ones
  matmul accumulates column sums into a persistent [1, 512] PSUM bank; a
  single tensor_reduce after the loop yields the core's scalar.

Each module is compiled once per process and relaunched for repeat calls.
"""

import threading
from contextlib import ExitStack

import numpy as np
import ml_dtypes

import concourse.bass as bass
import concourse.tile as tile
import concourse.mybir as mybir

N = 8192
D = 128
NC = 8
RPC = N // NC          # rows per core
NRT = RPC // 128       # 128-row tiles per core
CW = 1024              # compute chunk width (columns)
NK = N // CW           # chunks per row-tile
MW_BUFS = 3

F32 = mybir.dt.float32
BF16 = mybir.dt.bfloat16
AF = mybir.ActivationFunctionType
ALU = mybir.AluOpType


def _install_drain_patch():
    """The walrus build in this container rejects more than a couple of sem
    waits on one instruction; the Tile tail drain carries one wait per live
    semaphore. Emit them as individual sync-engine WAIT instructions."""
    if getattr(tile.TileContext, "_drain_patched", False):
        return

    def _patched(self, tick_clock, wait_clock):
        nc = self.nc
        carrier = nc.sync.nop()
        wait_clock.add_sem_waits(
            carrier.ins, tile.ScopedClock({None: tick_clock.global_clock})
        )
        si = carrier.ins.sync_info
        waits = list(si.on_wait or []) if si is not None else []
        if si is not None:
            si.on_wait = []
        handles = {}
        for h in self.sems.allocated().values():
            handles[getattr(h, "name", None) or getattr(h, "ant_name", None)] = h
        for w in waits:
            nc.sync.wait_ge(handles[w.ant_name], w.wait_value)
        nc.sync.drain()
        nc.all_engine_barrier()
        popped = nc._tile_sem_poison_stack.pop()
        assert popped is self._sem_poison
        nc.clear_and_free_semaphores(list(self.sems.allocated().values()))
        nc.all_engine_barrier()

    tile.TileContext._drain_and_barrier = _patched
    tile.TileContext._drain_patched = True


def _legalize_waits(nc, maxw=1):
    """Hoist excess per-instruction sem waits onto standalone EventSemaphore
    carriers just before the instruction on the same engine (wait A; wait B;
    inst  ==  inst waiting on A AND B)."""
    for fn in nc.m.functions:
        for blk in fn.blocks:
            insts = list(blk.instructions)
            new = []
            for inst in insts:
                si = inst.sync_info
                waits = list(si.on_wait) if si is not None and si.on_wait else []
                if len(waits) > maxw:
                    for i, w in enumerate(waits[:-maxw]):
                        new.append(mybir.InstEventSemaphore(
                            name=f"{inst.name}_hw{i}",
                            engine=inst.engine,
                            ins=[],
                            outs=[],
                            sync_info=mybir.SyncInfo(on_wait=[w], on_update=[]),
                        ))
                    si.on_wait = waits[-maxw:]
                new.append(inst)
            if len(new) != len(insts):
                blk.instructions[:] = new


def build_core_module(c: int, iters: int = 1, mw_dma_eng: str = "sync",
                      mw_bufs: int = MW_BUFS, psum_bufs: int = 3,
                      tbuf_bufs: int = 4, prod_fd: int = 1024,
                      gp_mod: int = 0, ablate: frozenset = frozenset()) -> bass.Bass:
    _install_drain_patch()
    nc = bass.Bass("TRN2", target_bir_lowering=False, debug=False)

    xs = nc.dram_tensor("xs", [RPC, D], F32, kind="ExternalInput").ap()
    y = nc.dram_tensor("y", [N, D], F32, kind="ExternalInput").ap()
    mws = nc.dram_tensor("mws", [RPC, 2 * N], BF16, kind="ExternalInput").ap()
    out = nc.dram_tensor("out", [1, 1], F32, kind="ExternalOutput").ap()

    ident_np = np.eye(128, dtype=np.float32)
    tri_np = (np.arange(128)[None, :] >= np.arange(128)[:, None]).astype(np.float32)
    ident_dram = nc.inline_tensor(ident_np, name="ident").ap()
    tri_dram = nc.inline_tensor(tri_np, name="tri").ap()

    with tile.TileContext(nc) as tc, ExitStack() as ctx:
        consts = ctx.enter_context(tc.tile_pool(name="consts", bufs=1))
        pre = ctx.enter_context(tc.tile_pool(name="pre", bufs=2))
        smalls = ctx.enter_context(tc.tile_pool(name="smalls", bufs=4))
        persist = ctx.enter_context(tc.tile_pool(name="persist", bufs=1))
        mp = ctx.enter_context(tc.tile_pool(name="mp", bufs=mw_bufs))
        wp = ctx.enter_context(tc.tile_pool(name="wp", bufs=mw_bufs))
        tb = ctx.enter_context(tc.tile_pool(name="tb", bufs=tbuf_bufs))
        prodw = ctx.enter_context(tc.tile_pool(name="prodw", bufs=2))
        psum_mm = ctx.enter_context(tc.tile_pool(name="psum_mm", bufs=psum_bufs,
                                                 space="PSUM"))
        psum_tr = ctx.enter_context(tc.tile_pool(name="psum_tr", bufs=1, space="PSUM"))
        psum_cs = ctx.enter_context(tc.tile_pool(name="psum_cs", bufs=1, space="PSUM"))

        ident_sb = consts.tile([128, 128], BF16, tag="ident")
        identf = pre.tile([128, 128], F32, tag="identf")
        nc.sync.dma_start(out=identf, in_=ident_dram)
        nc.vector.tensor_copy(out=ident_sb, in_=identf)
        tri_sb = consts.tile([128, 128], F32, tag="tri")
        nc.sync.dma_start(out=tri_sb, in_=tri_dram)

        ones_col = consts.tile([128, 1], BF16, tag="ones_col")
        nc.vector.memset(ones_col, 1.0)
        onesT1 = consts.tile([1, 128], BF16, tag="onesT1")
        nc.vector.memset(onesT1, 1.0)
        zrow = consts.tile([1, 512], BF16, tag="zrow")
        nc.vector.memset(zrow, 0.0)

        colsum = psum_cs.tile([1, 512], F32, tag="colsum")
        nc.tensor.matmul(colsum, onesT1[:, 0:1], zrow, start=True, stop=True)

        def prep_group(src_ap, g, dsts, negate):
            """Normalize + transpose rows [g*1024, (g+1)*1024) of src into
            eight [128,128] bf16 destination slices dsts[b]."""
            big = pre.tile([128, 8, 128], F32, tag="big")
            nc.sync.dma_start(
                out=big,
                in_=src_ap[g * 1024 : (g + 1) * 1024, :].rearrange(
                    "(b p) d -> p b d", p=128
                ),
            )
            sq = pre.tile([128, 8, 128], F32, tag="sq")
            nc.vector.tensor_mul(out=sq, in0=big, in1=big)
            n28 = smalls.tile([128, 8], F32, tag="n28")
            nc.vector.tensor_reduce(
                out=n28, in_=sq, axis=mybir.AxisListType.X, op=ALU.add
            )
            # rn = n2^(-1/2) via exp(-0.5*ln(n2))
            nc.scalar.activation(out=n28, in_=n28, func=AF.Ln)
            rn8 = smalls.tile([128, 8], F32, tag="rn8")
            nc.scalar.activation(out=rn8, in_=n28, func=AF.Exp, scale=-0.5)
            for b in range(8):
                hat = pre.tile([128, 128], BF16, tag="hat")
                if negate:
                    # x_hat = -2 * x / ||x||  so the PE produces v = -2S
                    nc.vector.tensor_scalar(
                        out=hat, in0=big[:, b, :], scalar1=rn8[:, b : b + 1],
                        scalar2=-2.0, op0=ALU.mult, op1=ALU.mult,
                    )
                else:
                    nc.vector.tensor_scalar_mul(
                        out=hat, in0=big[:, b, :], scalar1=rn8[:, b : b + 1]
                    )
                pt = psum_tr.tile([128, 128], BF16, tag="pt")
                nc.tensor.matmul(pt, hat, ident_sb, is_transpose=True)
                nc.vector.tensor_copy(out=dsts[b], in_=pt)

        xT = [persist.tile([128, 128], BF16, tag=f"xT{rt}", name=f"xT{rt}")
              for rt in range(NRT)]
        prep_group(xs, 0, xT, negate=True)

        yT = [persist.tile([128, CW], BF16, tag=f"yT{k}", name=f"yT{k}")
              for k in range(NK)]
        for k in range(NK):
            prep_group(y, k, [yT[k][:, b * 128 : (b + 1) * 128] for b in range(8)],
                       negate=False)

        def prod_phase(w_row, tbuf):
            """prod = w .* softplus (bf16 TT, 2x), column sums into PSUM via
            1-partition ones matmuls. Emitted one row-tile late so the PE's
            colsum matmuls (which wait on the end of a row's pipeline) queue
            behind the NEXT row's v matmuls rather than ahead of them.
            Every gp_mod-th chunk's multiply runs on the otherwise-idle GPSIMD
            engine to take load off the DVE; DVE chunks are split at prod_fd
            to shrink the post-op pipeline-drain penalty."""
            if "prod" in ablate:
                return
            for k in range(NK):
                prod = prodw.tile([128, CW], BF16, tag="prod")
                if gp_mod > 0 and k % gp_mod == 0:
                    nc.gpsimd.tensor_mul(
                        out=prod, in0=w_row[:, k * CW : (k + 1) * CW],
                        in1=tbuf[:, k * CW : (k + 1) * CW],
                    )
                else:
                    for s in range(0, CW, prod_fd):
                        nc.vector.tensor_mul(
                            out=prod[:, s : s + prod_fd],
                            in0=w_row[:, k * CW + s : k * CW + s + prod_fd],
                            in1=tbuf[:, k * CW + s : k * CW + s + prod_fd],
                        )
                if "colsum" in ablate:
                    continue
                for s in range(0, CW, 512):
                    nc.tensor.matmul(colsum, ones_col, prod[:, s : s + 512],
                                     start=False, stop=True,
                                     skip_group_check=True)

        mw_static = None
        if "dma" in ablate:
            m0 = mp.tile([128, N], BF16, tag="ms0", name="ms0")
            nc.sync.dma_start(out=m0, in_=mws[0:128, 0:N])
            w0 = wp.tile([128, N], BF16, tag="ws0", name="ws0")
            nc.sync.dma_start(out=w0, in_=mws[0:128, N : 2 * N])
            mw_static = (m0, w0)

        def main_loop():
            # prods run TWO row-tiles behind: on the in-order DVE queue they
            # otherwise sit between ln(g-1) (their dependency) and t(g),
            # serializing each row as prods+t+exp+ln instead of ACT-only.
            pend = []
            for g in range(NRT):
                if mw_static is not None:
                    m_row, w_row = mw_static
                else:
                    m_row = mp.tile([128, N], BF16, tag="m")
                    getattr(nc, mw_dma_eng).dma_start(
                        out=m_row, in_=mws[g * 128 : (g + 1) * 128, 0:N]
                    )
                    w_row = wp.tile([128, N], BF16, tag="w")
                    getattr(nc, mw_dma_eng).dma_start(
                        out=w_row, in_=mws[g * 128 : (g + 1) * 128, N : 2 * N]
                    )

                tbuf = tb.tile([128, N], BF16, tag="tbuf")

                # ---- t for unmasked chunks (k >= g) via PE + STT ----
                k_lo = NK if "t" in ablate else g
                for k in range(k_lo, NK):
                    v = psum_mm.tile([128, CW], F32, tag="v")
                    if k == g:
                        # cols < c*128 are fully masked: zero-fill via a
                        # 1-partition matmul so t = (0+1)*m = m there
                        cmask = c * 128
                        for s in range(0, cmask, 512):
                            e = min(cmask, s + 512)
                            nc.tensor.matmul(v[:, s:e], onesT1, zrow[:, 0 : e - s],
                                             start=True, stop=True)
                        for s in range(0, CW, 512):
                            e = s + 512
                            s0 = max(s, cmask)
                            if s0 < e:
                                nc.tensor.matmul(
                                    v[:, s0:e], xT[g],
                                    yT[k][:, s0:e], start=True, stop=True,
                                )
                        # triangular mask on 128-block c (v = -2S there; below
                        # the diagonal force v=0 so t = m)
                        nc.vector.tensor_mul(
                            out=v[:, cmask : cmask + 128],
                            in0=v[:, cmask : cmask + 128],
                            in1=tri_sb,
                        )
                    else:
                        for s in range(0, CW, 512):
                            nc.tensor.matmul(
                                v[:, s : s + 512], xT[g],
                                yT[k][:, s : s + 512], start=True, stop=True,
                            )
                    nc.vector.scalar_tensor_tensor(
                        out=tbuf[:, k * CW : (k + 1) * CW], in0=v, scalar=1.0,
                        in1=m_row[:, k * CW : (k + 1) * CW],
                        op0=ALU.add, op1=ALU.mult,
                    )

                if len(pend) >= 2:
                    prod_phase(*pend.pop(0))
                pend.append((w_row, tbuf))

                # ---- row-batched ACT: exp(prefix from m), exp(suffix), ln ----
                if g > 0:
                    nc.scalar.activation(out=tbuf[:, 0 : g * CW],
                                         in_=m_row[:, 0 : g * CW],
                                         func=AF.Exp, scale=1.0)
                nc.scalar.activation(out=tbuf[:, g * CW :], in_=tbuf[:, g * CW :],
                                     func=AF.Exp, scale=1.0)
                nc.scalar.activation(out=tbuf, in_=tbuf, func=AF.Ln, bias=1.0)
            for p in pend:
                prod_phase(*p)

        if iters == 1:
            main_loop()
        else:
            # timing mode: repeat the streaming loop on-device so dispatch
            # overhead amortizes out of wall-clock measurements; branch hints
            # keep the large body's back-edge IRAM-resident
            with tc.For_i(0, iters, 1, hint_engines=(
                mybir.EngineType.DVE, mybir.EngineType.Activation,
                mybir.EngineType.PE, mybir.EngineType.SP,
            )):
                main_loop()

        total = smalls.tile([1, 1], F32, tag="total")
        nc.vector.tensor_reduce(
            out=total, in_=colsum, axis=mybir.AxisListType.X, op=ALU.add
        )
        nc.sync.dma_start(out=out, in_=total)

    _legalize_waits(nc)
    return nc


class CoreRunner:
    """One jitted bass_exec per (module, device); compiled once, relaunchable."""

    def __init__(self, nc, device):
        import jax
        from concourse import bass2jax

        bass2jax.install_neuronx_cc_hook()
        self.nc = nc
        self.device = device
        self.partition_name = (
            nc.partition_id_tensor.name if nc.partition_id_tensor is not None else None
        )
        in_names, out_names, out_avals = [], [], []
        self.out_shapes = []
        for alloc in nc.m.functions[0].allocations:
            if not isinstance(alloc, mybir.MemoryLocationSet):
                continue
            name = alloc.memorylocations[0].name
            if alloc.kind == "ExternalInput":
                if name != self.partition_name:
                    in_names.append(name)
            elif alloc.kind == "ExternalOutput":
                out_names.append(name)
                shape = tuple(alloc.tensor_shape)
                dtype = mybir.dt.np(alloc.dtype)
                out_avals.append(jax.core.ShapedArray(shape, dtype))
                self.out_shapes.append((shape, dtype))
        self.in_names = in_names
        self.out_names = out_names
        n_params, n_outs = len(in_names), len(out_names)
        extra = [self.partition_name] if self.partition_name else []
        all_in_names = tuple(in_names + out_names + extra)
        donate = tuple(range(n_params, n_params + n_outs))
        out_avals_t = tuple(out_avals)

        def _body(*args):
            outs = bass2jax._bass_exec_p.bind(
                *args,
                out_avals=out_avals_t,
                in_names=all_in_names,
                out_names=tuple(out_names),
                lowering_input_output_aliases=(),
                sim_require_finite=True,
                sim_require_nnan=True,
                nc=nc,
            )
            return tuple(outs)

        self.jitted = jax.jit(_body, donate_argnums=donate, keep_unused=True)
        self._dev_inputs = None
        self._pid = []

    def upload(self, in_map, core_id=0):
        import jax

        self._dev_inputs = [
            jax.device_put(np.ascontiguousarray(in_map[name]), self.device)
            for name in self.in_names
        ]
        self._pid = (
            [jax.device_put(np.array([[core_id]], np.uint32), self.device)]
            if self.partition_name
            else []
        )

    def launch(self):
        import jax

        zeros = [
            jax.device_put(np.zeros(shape, dtype), self.device)
            for shape, dtype in self.out_shapes
        ]
        return self.jitted(*self._dev_inputs, *zeros, *self._pid)

    def gather(self, outs):
        return {name: np.asarray(o) for name, o in zip(self.out_names, outs)}


_runners = None
_runner_lock = threading.Lock()


def get_runners():
    global _runners
    with _runner_lock:
        if _runners is None:
            import jax

            devs = jax.devices()
            assert len(devs) >= NC, f"need {NC} devices, have {len(devs)}"
            _runners = [CoreRunner(build_core_module(c), devs[c]) for c in range(NC)]
        return _runners


def shard_inputs(x, y, m, w):
    """Row-tiles (128 rows each) are dealt round-robin: core c gets global
    tiles {8g + c}, so every core sees the same balanced mix of masked /
    diagonal / unmasked column chunks (the diagonal of tile 8g+c falls in
    column chunk g for every core). m and w are cast to bf16 and concatenated
    per row-tile so each [128, 16384] tile is one contiguous 4 MB DMA."""
    bf16 = ml_dtypes.bfloat16
    xr = x.reshape(NC * NRT, 128, D)
    mr = np.asarray(m, dtype=np.float32).astype(bf16).reshape(NC * NRT, 128, N)
    wr = np.asarray(w, dtype=np.float32).astype(bf16).reshape(NC * NRT, 128, N)
    maps = []
    for c in range(NC):
        mw = np.concatenate([mr[c::NC], wr[c::NC]], axis=2).reshape(RPC, 2 * N)
        maps.append({
            "xs": xr[c::NC].reshape(RPC, D),
            "y": y,
            "mws": mw,
        })
    return maps


def kernel(x, y, m, w):
    x = np.asarray(x, dtype=np.float32)
    y = np.asarray(y, dtype=np.float32)
    m = np.asarray(m, dtype=np.float32)
    w = np.asarray(w, dtype=np.float32)
    assert x.shape == (N, D) and y.shape == (N, D)
    assert m.shape == (N, N) and w.shape == (N, N)
    runners = get_runners()
    maps = shard_inputs(x, y, m, w)
    for c, r in enumerate(runners):
        r.upload(maps[c], core_id=c)
    handles = [r.launch() for r in runners]
    results = [r.gather(h) for r, h in zip(runners, handles)]
    total = np.float64(0.0)
    for res in results:
        total += np.float64(res["out"].ravel()[0])
    return np.float32(total)


# revision 33
# speedup vs baseline: 1.3822x; 1.1461x over previous
"""Binomial deviance loss on 8 Trainium2 NeuronCores (Bass/Tile, SPMD-free).

loss = sum(w * log1p(exp(-ALPHA*(S-BETA)*m))),  S = triu(cosine(x_i, y_j)),
ALPHA = 2.0, BETA = 0.5  ->  t = (1 - 2*S)*m and loss = sum(w * softplus(t)).

Sharding: the 64 128-row tiles of x/m/w are dealt round-robin across the 8
NeuronCores (core c gets tiles {8g+c}), which balances the triangular-mask
work exactly; y is replicated. m and w are converted to bf16 on the host and
concatenated row-tile-wise into one [RPC, 2N] array so each 128-row tile is
a single 4 MB DMA (m in cols [0,N), w in cols [N,2N)) — this halves the HBM
traffic, which is the roofline for this problem. Each core returns a scalar
partial loss [1, 1]; the host sums the 8 scalars.

Per core c, local row-tile g (global 128-row tile G = 8g+c), chunks of 1024:
  x_hat = x * (-2/||x||)   (so the PE output is v = -2S)
  y_hat = y * (1/||y||)
  chunk k < g : fully masked -> t = m (ACT exp reads m directly)
  chunk k > g : v = PE(x_hatT, y_hatT) in fp32 PSUM;  t = (v+1)*m   (DVE STT)
  chunk k == g: cols < c*128 zero-filled in PSUM by a 1-partition matmul,
                128-block c gets the triangular mask (DVE TT with tri), then
                t = (v+1)*m as above.
  ACT (row-batched, 3 instructions per row-tile):
      tbuf[:, :g*1024]  = exp(m_prefix)        (masked part, from m directly)
      tbuf[:, g*1024:]  = exp(tbuf suffix)     (in place)
      tbuf              = ln(tbuf + 1)         (in place -> softplus)
  prod = w .* tbuf per chunk (bf16 TT, 2x mode), then a 1-partition ones
  matmul accumulates column sums into a persistent [1, 512] PSUM bank; a
  single tensor_reduce after the loop yields the core's scalar.

Each module is compiled once per process and relaunched for repeat calls.
"""

import threading
from contextlib import ExitStack

import numpy as np
import ml_dtypes

import concourse.bass as bass
import concourse.tile as tile
import concourse.mybir as mybir

N = 8192
D = 128
NC = 8
RPC = N // NC          # rows per core
NRT = RPC // 128       # 128-row tiles per core
CW = 1024              # compute chunk width (columns)
NK = N // CW           # chunks per row-tile
MW_BUFS = 3

F32 = mybir.dt.float32
BF16 = mybir.dt.bfloat16
AF = mybir.ActivationFunctionType
ALU = mybir.AluOpType


def _install_drain_patch():
    """The walrus build in this container rejects more than a couple of sem
    waits on one instruction; the Tile tail drain carries one wait per live
    semaphore. Emit them as individual sync-engine WAIT instructions."""
    if getattr(tile.TileContext, "_drain_patched", False):
        return

    def _patched(self, tick_clock, wait_clock):
        nc = self.nc
        carrier = nc.sync.nop()
        wait_clock.add_sem_waits(
            carrier.ins, tile.ScopedClock({None: tick_clock.global_clock})
        )
        si = carrier.ins.sync_info
        waits = list(si.on_wait or []) if si is not None else []
        if si is not None:
            si.on_wait = []
        handles = {}
        for h in self.sems.allocated().values():
            handles[getattr(h, "name", None) or getattr(h, "ant_name", None)] = h
        for w in waits:
            nc.sync.wait_ge(handles[w.ant_name], w.wait_value)
        nc.sync.drain()
        nc.all_engine_barrier()
        popped = nc._tile_sem_poison_stack.pop()
        assert popped is self._sem_poison
        nc.clear_and_free_semaphores(list(self.sems.allocated().values()))
        nc.all_engine_barrier()

    tile.TileContext._drain_and_barrier = _patched
    tile.TileContext._drain_patched = True


def _legalize_waits(nc, maxw=1):
    """Hoist excess per-instruction sem waits onto standalone EventSemaphore
    carriers just before the instruction on the same engine (wait A; wait B;
    inst  ==  inst waiting on A AND B)."""
    for fn in nc.m.functions:
        for blk in fn.blocks:
            insts = list(blk.instructions)
            new = []
            for inst in insts:
                si = inst.sync_info
                waits = list(si.on_wait) if si is not None and si.on_wait else []
                if len(waits) > maxw:
                    for i, w in enumerate(waits[:-maxw]):
                        new.append(mybir.InstEventSemaphore(
                            name=f"{inst.name}_hw{i}",
                            engine=inst.engine,
                            ins=[],
                            outs=[],
                            sync_info=mybir.SyncInfo(on_wait=[w], on_update=[]),
                        ))
                    si.on_wait = waits[-maxw:]
                new.append(inst)
            if len(new) != len(insts):
                blk.instructions[:] = new


def build_core_module(c: int, iters: int = 1, mw_dma_eng: str = "sync",
                      mw_bufs: int = MW_BUFS, psum_bufs: int = 3,
                      tbuf_bufs: int = 3, prod_fd: int = 1024,
                      gp_mod: int = 0, reverse_g: bool = False,
                      suf_split: int = 1, row_order: str = "asc",
                      prefix_dma: bool = False,
                      ablate: frozenset = frozenset()) -> bass.Bass:
    _install_drain_patch()
    nc = bass.Bass("TRN2", target_bir_lowering=False, debug=False)

    xs = nc.dram_tensor("xs", [RPC, D], F32, kind="ExternalInput").ap()
    y = nc.dram_tensor("y", [N, D], F32, kind="ExternalInput").ap()
    mws = nc.dram_tensor("mws", [RPC, 2 * N], BF16, kind="ExternalInput").ap()
    out = nc.dram_tensor("out", [1, 1], F32, kind="ExternalOutput").ap()

    ident_np = np.eye(128, dtype=np.float32)
    tri_np = (np.arange(128)[None, :] >= np.arange(128)[:, None]).astype(np.float32)
    ident_dram = nc.inline_tensor(ident_np, name="ident").ap()
    tri_dram = nc.inline_tensor(tri_np, name="tri").ap()

    with tile.TileContext(nc) as tc, ExitStack() as ctx:
        consts = ctx.enter_context(tc.tile_pool(name="consts", bufs=1))
        pre = ctx.enter_context(tc.tile_pool(name="pre", bufs=1))
        smalls = ctx.enter_context(tc.tile_pool(name="smalls", bufs=4))
        persist = ctx.enter_context(tc.tile_pool(name="persist", bufs=1))
        mp = ctx.enter_context(tc.tile_pool(name="mp", bufs=2))
        wp = ctx.enter_context(tc.tile_pool(name="wp", bufs=2))
        tb = ctx.enter_context(tc.tile_pool(name="tb", bufs=tbuf_bufs))
        prodw = ctx.enter_context(tc.tile_pool(name="prodw", bufs=2))
        psum_mm = ctx.enter_context(tc.tile_pool(name="psum_mm", bufs=psum_bufs,
                                                 space="PSUM"))
        psum_tr = ctx.enter_context(tc.tile_pool(name="psum_tr", bufs=1, space="PSUM"))
        psum_cs = ctx.enter_context(tc.tile_pool(name="psum_cs", bufs=1, space="PSUM"))

        ident_sb = consts.tile([128, 128], BF16, tag="ident")
        identf = pre.tile([128, 128], F32, tag="identf")
        nc.sync.dma_start(out=identf, in_=ident_dram)
        nc.vector.tensor_copy(out=ident_sb, in_=identf)
        tri_sb = consts.tile([128, 128], F32, tag="tri")
        nc.sync.dma_start(out=tri_sb, in_=tri_dram)

        ones_col = consts.tile([128, 1], BF16, tag="ones_col")
        nc.vector.memset(ones_col, 1.0)
        onesT1 = consts.tile([1, 128], BF16, tag="onesT1")
        nc.vector.memset(onesT1, 1.0)
        zrow = consts.tile([1, 512], BF16, tag="zrow")
        nc.vector.memset(zrow, 0.0)

        colsum = psum_cs.tile([1, 512], F32, tag="colsum")
        nc.tensor.matmul(colsum, onesT1[:, 0:1], zrow, start=True, stop=True)

        def prep_group(src_ap, g, dsts, negate):
            """Normalize + transpose rows [g*1024, (g+1)*1024) of src into
            eight [128,128] bf16 destination slices dsts[b]."""
            big = pre.tile([128, 8, 128], F32, tag="big")
            nc.sync.dma_start(
                out=big,
                in_=src_ap[g * 1024 : (g + 1) * 1024, :].rearrange(
                    "(b p) d -> p b d", p=128
                ),
            )
            sq = pre.tile([128, 8, 128], F32, tag="sq")
            nc.vector.tensor_mul(out=sq, in0=big, in1=big)
            n28 = smalls.tile([128, 8], F32, tag="n28")
            nc.vector.tensor_reduce(
                out=n28, in_=sq, axis=mybir.AxisListType.X, op=ALU.add
            )
            # rn = n2^(-1/2) via exp(-0.5*ln(n2))
            nc.scalar.activation(out=n28, in_=n28, func=AF.Ln)
            rn8 = smalls.tile([128, 8], F32, tag="rn8")
            nc.scalar.activation(out=rn8, in_=n28, func=AF.Exp, scale=-0.5)
            for b in range(8):
                hat = pre.tile([128, 128], BF16, tag="hat")
                if negate:
                    # x_hat = -2 * x / ||x||  so the PE produces v = -2S
                    nc.vector.tensor_scalar(
                        out=hat, in0=big[:, b, :], scalar1=rn8[:, b : b + 1],
                        scalar2=-2.0, op0=ALU.mult, op1=ALU.mult,
                    )
                else:
                    nc.vector.tensor_scalar_mul(
                        out=hat, in0=big[:, b, :], scalar1=rn8[:, b : b + 1]
                    )
                pt = psum_tr.tile([128, 128], BF16, tag="pt")
                nc.tensor.matmul(pt, hat, ident_sb, is_transpose=True)
                nc.vector.tensor_copy(out=dsts[b], in_=pt)

        xT = [persist.tile([128, 128], BF16, tag=f"xT{rt}", name=f"xT{rt}")
              for rt in range(NRT)]
        prep_group(xs, 0, xT, negate=True)

        yT = [persist.tile([128, CW], BF16, tag=f"yT{k}", name=f"yT{k}")
              for k in range(NK)]
        for k in range(NK):
            prep_group(y, k, [yT[k][:, b * 128 : (b + 1) * 128] for b in range(8)],
                       negate=False)

        def prod_phase(w_row, tbuf):
            """prod = w .* softplus (bf16 TT, 2x), column sums into PSUM via
            1-partition ones matmuls. Emitted one row-tile late so the PE's
            colsum matmuls (which wait on the end of a row's pipeline) queue
            behind the NEXT row's v matmuls rather than ahead of them.
            Every gp_mod-th chunk's multiply runs on the otherwise-idle GPSIMD
            engine to take load off the DVE; DVE chunks are split at prod_fd
            to shrink the post-op pipeline-drain penalty."""
            if "prod" in ablate:
                return
            for k in range(NK):
                prod = prodw.tile([128, CW], BF16, tag="prod")
                if gp_mod > 0 and k % gp_mod == 0:
                    nc.gpsimd.tensor_mul(
                        out=prod, in0=w_row[:, k * CW : (k + 1) * CW],
                        in1=tbuf[:, k * CW : (k + 1) * CW],
                    )
                else:
                    for s in range(0, CW, prod_fd):
                        nc.vector.tensor_mul(
                            out=prod[:, s : s + prod_fd],
                            in0=w_row[:, k * CW + s : k * CW + s + prod_fd],
                            in1=tbuf[:, k * CW + s : k * CW + s + prod_fd],
                        )
                if "colsum" in ablate:
                    continue
                for s in range(0, CW, 512):
                    nc.tensor.matmul(colsum, ones_col, prod[:, s : s + 512],
                                     start=False, stop=True,
                                     skip_group_check=True)

        def t_phase(g, m_row, tbuf):
            """v = -2S via PE into fp32 PSUM, then t = (v+1)*m per chunk."""
            if "t" in ablate:
                return
            for k in range(g, NK):
                v = psum_mm.tile([128, CW], F32, tag="v")
                if k == g:
                    # cols < c*128 are fully masked: zero-fill via a
                    # 1-partition matmul so t = (0+1)*m = m there
                    cmask = c * 128
                    for s in range(0, cmask, 512):
                        e = min(cmask, s + 512)
                        nc.tensor.matmul(v[:, s:e], onesT1, zrow[:, 0 : e - s],
                                         start=True, stop=True)
                    for s in range(0, CW, 512):
                        e = s + 512
                        s0 = max(s, cmask)
                        if s0 < e:
                            nc.tensor.matmul(
                                v[:, s0:e], xT[g],
                                yT[k][:, s0:e], start=True, stop=True,
                            )
                    # triangular mask on 128-block c (v = -2S there; below
                    # the diagonal force v=0 so t = m)
                    nc.vector.tensor_mul(
                        out=v[:, cmask : cmask + 128],
                        in0=v[:, cmask : cmask + 128],
                        in1=tri_sb,
                    )
                else:
                    for s in range(0, CW, 512):
                        nc.tensor.matmul(
                            v[:, s : s + 512], xT[g],
                            yT[k][:, s : s + 512], start=True, stop=True,
                        )
                nc.vector.scalar_tensor_tensor(
                    out=tbuf[:, k * CW : (k + 1) * CW], in0=v, scalar=1.0,
                    in1=m_row[:, k * CW : (k + 1) * CW],
                    op0=ALU.add, op1=ALU.mult,
                )

        # Row 0 (all-unmasked, no exp-prefix to hide behind) lives in
        # persistent tiles; its t-phase is recomputed at the TAIL of each
        # iteration — before the ln-gated tail prods — so the next
        # iteration's ACT starts with zero bubble at the loop back-edge.
        m0 = persist.tile([128, N], BF16, tag="m0", name="m0")
        w0 = persist.tile([128, N], BF16, tag="w0", name="w0")
        tb0 = persist.tile([128, N], BF16, tag="tb0", name="tb0")

        def row0_load_and_t():
            getattr(nc, mw_dma_eng).dma_start(out=m0, in_=mws[0:128, 0:N])
            getattr(nc, mw_dma_eng).dma_start(out=w0, in_=mws[0:128, N : 2 * N])
            t_phase(0, m0, tb0)

        row0_load_and_t()  # prologue

        def main_loop():
            # prods run TWO row-tiles behind: on the in-order DVE queue they
            # otherwise sit between ln(g-1) (their dependency) and t(g),
            # serializing each row as prods+t+exp+ln instead of ACT-only.
            pend = []
            for g in range(NRT):
                # prods of row g-2 go first in each section: their deps
                # (ln(g-2)) are long satisfied, and emitting them before this
                # section's DMAs lets the m/w pools run at 2 buffers
                if len(pend) >= 2:
                    prod_phase(*pend.pop(0))
                if g == 0:
                    m_row, w_row, tbuf = m0, w0, tb0
                else:
                    m_row = mp.tile([128, N], BF16, tag="m")
                    getattr(nc, mw_dma_eng).dma_start(
                        out=m_row, in_=mws[g * 128 : (g + 1) * 128, 0:N]
                    )
                    w_row = wp.tile([128, N], BF16, tag="w")
                    getattr(nc, mw_dma_eng).dma_start(
                        out=w_row, in_=mws[g * 128 : (g + 1) * 128, N : 2 * N]
                    )
                    tbuf = tb.tile([128, N], BF16, tag="tbuf")
                    t_phase(g, m_row, tbuf)
                pend.append((w_row, tbuf))

                # ---- row-batched ACT: exp(prefix from m), exp(suffix), ln ----
                if prefix_dma:
                    # copy the fully-masked prefix (t = m) into tbuf over the
                    # idle SWDGE queue, then one full-row exp on ACT
                    if g > 0:
                        nc.sync.dma_start(out=tbuf[:, 0 : g * CW],
                                          in_=m_row[:, 0 : g * CW])
                    nc.scalar.activation(out=tbuf, in_=tbuf, func=AF.Exp,
                                         scale=1.0)
                else:
                    if g > 0:
                        nc.scalar.activation(out=tbuf[:, 0 : g * CW],
                                             in_=m_row[:, 0 : g * CW],
                                             func=AF.Exp, scale=1.0)
                    nsuf = NK - g
                    per = max(1, nsuf // suf_split)
                    s0 = g * CW
                    while s0 < N:
                        s1 = min(N, s0 + per * CW) if s0 + per * CW < N else N
                        nc.scalar.activation(out=tbuf[:, s0:s1],
                                             in_=tbuf[:, s0:s1],
                                             func=AF.Exp, scale=1.0)
                        s0 = s1
                nc.scalar.activation(out=tbuf, in_=tbuf, func=AF.Ln, bias=1.0)
            # tail: reload + recompute row 0's t for the next iteration BEFORE
            # the ln-gated tail prods, so the loop back-edge has no ACT bubble
            row0_load_and_t()
            for p in pend:
                prod_phase(*p)

        if iters == 1:
            main_loop()
        else:
            # timing mode: repeat the streaming loop on-device so dispatch
            # overhead amortizes out of wall-clock measurements; branch hints
            # keep the large body's back-edge IRAM-resident
            with tc.For_i(0, iters, 1, hint_engines=(
                mybir.EngineType.DVE, mybir.EngineType.Activation,
                mybir.EngineType.PE, mybir.EngineType.SP,
            )):
                main_loop()

        total = smalls.tile([1, 1], F32, tag="total")
        nc.vector.tensor_reduce(
            out=total, in_=colsum, axis=mybir.AxisListType.X, op=ALU.add
        )
        nc.sync.dma_start(out=out, in_=total)

    _legalize_waits(nc)
    return nc


class CoreRunner:
    """One jitted bass_exec per (module, device); compiled once, relaunchable."""

    def __init__(self, nc, device):
        import jax
        from concourse import bass2jax

        bass2jax.install_neuronx_cc_hook()
        self.nc = nc
        self.device = device
        self.partition_name = (
            nc.partition_id_tensor.name if nc.partition_id_tensor is not None else None
        )
        in_names, out_names, out_avals = [], [], []
        self.out_shapes = []
        for alloc in nc.m.functions[0].allocations:
            if not isinstance(alloc, mybir.MemoryLocationSet):
                continue
            name = alloc.memorylocations[0].name
            if alloc.kind == "ExternalInput":
                if name != self.partition_name:
                    in_names.append(name)
            elif alloc.kind == "ExternalOutput":
                out_names.append(name)
                shape = tuple(alloc.tensor_shape)
                dtype = mybir.dt.np(alloc.dtype)
                out_avals.append(jax.core.ShapedArray(shape, dtype))
                self.out_shapes.append((shape, dtype))
        self.in_names = in_names
        self.out_names = out_names
        n_params, n_outs = len(in_names), len(out_names)
        extra = [self.partition_name] if self.partition_name else []
        all_in_names = tuple(in_names + out_names + extra)
        donate = tuple(range(n_params, n_params + n_outs))
        out_avals_t = tuple(out_avals)

        def _body(*args):
            outs = bass2jax._bass_exec_p.bind(
                *args,
                out_avals=out_avals_t,
                in_names=all_in_names,
                out_names=tuple(out_names),
                lowering_input_output_aliases=(),
                sim_require_finite=True,
                sim_require_nnan=True,
                nc=nc,
            )
            return tuple(outs)

        self.jitted = jax.jit(_body, donate_argnums=donate, keep_unused=True)
        self._dev_inputs = None
        self._pid = []

    def upload(self, in_map, core_id=0):
        import jax

        self._dev_inputs = [
            jax.device_put(np.ascontiguousarray(in_map[name]), self.device)
            for name in self.in_names
        ]
        self._pid = (
            [jax.device_put(np.array([[core_id]], np.uint32), self.device)]
            if self.partition_name
            else []
        )

    def launch(self):
        import jax

        zeros = [
            jax.device_put(np.zeros(shape, dtype), self.device)
            for shape, dtype in self.out_shapes
        ]
        return self.jitted(*self._dev_inputs, *zeros, *self._pid)

    def gather(self, outs):
        return {name: np.asarray(o) for name, o in zip(self.out_names, outs)}


_runners = None
_runner_lock = threading.Lock()


def get_runners():
    global _runners
    with _runner_lock:
        if _runners is None:
            import jax

            devs = jax.devices()
            assert len(devs) >= NC, f"need {NC} devices, have {len(devs)}"
            _runners = [CoreRunner(build_core_module(c), devs[c]) for c in range(NC)]
        return _runners


def shard_inputs(x, y, m, w):
    """Row-tiles (128 rows each) are dealt round-robin: core c gets global
    tiles {8g + c}, so every core sees the same balanced mix of masked /
    diagonal / unmasked column chunks (the diagonal of tile 8g+c falls in
    column chunk g for every core). m and w are cast to bf16 and concatenated
    per row-tile so each [128, 16384] tile is one contiguous 4 MB DMA."""
    bf16 = ml_dtypes.bfloat16
    xr = x.reshape(NC * NRT, 128, D)
    mr = np.asarray(m, dtype=np.float32).astype(bf16).reshape(NC * NRT, 128, N)
    wr = np.asarray(w, dtype=np.float32).astype(bf16).reshape(NC * NRT, 128, N)
    maps = []
    for c in range(NC):
        mw = np.concatenate([mr[c::NC], wr[c::NC]], axis=2).reshape(RPC, 2 * N)
        maps.append({
            "xs": xr[c::NC].reshape(RPC, D),
            "y": y,
            "mws": mw,
        })
    return maps


def kernel(x, y, m, w):
    x = np.asarray(x, dtype=np.float32)
    y = np.asarray(y, dtype=np.float32)
    m = np.asarray(m, dtype=np.float32)
    w = np.asarray(w, dtype=np.float32)
    assert x.shape == (N, D) and y.shape == (N, D)
    assert m.shape == (N, N) and w.shape == (N, N)
    runners = get_runners()
    maps = shard_inputs(x, y, m, w)
    for c, r in enumerate(runners):
        r.upload(maps[c], core_id=c)
    handles = [r.launch() for r in runners]
    results = [r.gather(h) for r, h in zip(runners, handles)]
    total = np.float64(0.0)
    for res in results:
        total += np.float64(res["out"].ravel()[0])
    return np.float32(total)
